# revision 11
# baseline (speedup 1.0000x reference)
"""TRN2 Bass kernel for VQ codebook lookup (nn_BrainEncoder).

Contract: kernel(**inputs) takes FULL inputs
    z_e:      (32, 512, 256) f32   (b, d, t)
    codebook: (8192, 512) f32      (K, d)
returns (z_q_out (32,512,256) f32, vq_loss f32 scalar, perplexity f32 scalar)
exactly like the reference nn.Module.

Distribution: data-parallel over the flattened token dim N = 32*256 = 8192
across 8 NeuronCores (1024 tokens each); the codebook is replicated.

Per-core device kernel:
  - PE: dist'[n,k] = -2*x.c (bf16 matmuls) + |c_k|^2 + unique-k epsilon
        (one extra f32r matmul row) accumulated in PSUM fp32.
  - ACT: negate-copy PSUM -> SBUF scores.
  - DVE: top-8 scan + max_index -> candidate indices per token.
  - indirect-DMA gather of top-4 candidate rows (+their |c|^2).
  - exact fp32 rescore (DVE fused mult+accum) -> winner = exact fp32 argmin,
    ties to lowest k (matches jnp.argmin; the coarse bf16 pass only
    shortlists, the winner is decided at full fp32 precision).
  - winner row gather; straight-through out = z + (z_q - z) elementwise in
    fp32 (bit-identical formula to the reference); loss partials via ACT
    Square+accumulate; PE transposes the output back to d-major.
Host: shard/unshard, sum of per-core loss partials, bincount of the 8192
winner indices -> perplexity (the tiny O(K) scalar tail of the op).
"""

import numpy as np

ALPHA = 1.0
BETA = 0.25
B, EMBED_DIM, T = 32, 512, 256
K = 8192
D = 512
N_CORES = 8
NTILES = 8
NCHUNK = 16
NCAND = 4
CBW = 520

_CACHE = {}


def _build(n_loop=0, n_cores=N_CORES, stages=3):
    import concourse.bass as bass
    import concourse.mybir as mybir
    from concourse import bacc
    from concourse.tile import TileContext
    from concourse.masks import make_identity

    F32 = mybir.dt.float32
    F32R = mybir.dt.float32r
    BF16 = mybir.dt.bfloat16
    U32 = mybir.dt.uint32
    AX = mybir.AxisListType.X
    ALU = mybir.AluOpType
    ACTF = mybir.ActivationFunctionType

    nc = bacc.Bacc("TRN2", target_bir_lowering=False, debug=False,
                   num_devices=n_cores)
    xt_d = nc.declare_dram_parameter("xt", [512, 1024], BF16, isOutput=False)
    x_d = nc.declare_dram_parameter("x", [1024, 512], F32, isOutput=False)
    ct_d = nc.declare_dram_parameter("ct", [512, K], BF16, isOutput=False)
    crow_d = nc.declare_dram_parameter("crow", [4, K], F32, isOutput=False)
    ones_d = nc.declare_dram_parameter("ones", [4, 128], F32, isOutput=False)
    cbn_d = nc.declare_dram_parameter("cbn", [K, CBW], F32, isOutput=False)
    zqt_d = nc.declare_dram_parameter("zqt", [512, 1024], F32, isOutput=True)
    widx_d = nc.declare_dram_parameter("widx", [128, NTILES], U32, isOutput=True)
    lossp_d = nc.declare_dram_parameter("lossp", [128, NTILES], F32, isOutput=True)

    with TileContext(nc) as tc:
        with tc.tile_pool(name="big", bufs=1) as big, \
             tc.tile_pool(name="sc", bufs=2) as scp, \
             tc.tile_pool(name="work", bufs=2) as work, \
             tc.tile_pool(name="small", bufs=3) as small, \
             tc.tile_pool(name="psum", bufs=8, space="PSUM") as psum:
            ct_s = big.tile([128, 4, K], BF16, tag="ct")
            for d in range(4):
                nc.sync.dma_start(out=ct_s[:, d, :], in_=ct_d[d*128:(d+1)*128, :])
            crow_s = big.tile([4, K], F32R, tag="crow")
            nc.gpsimd.dma_start(out=crow_s[:, :], in_=crow_d[:, :])
            ones_s = big.tile([4, 128], F32R, tag="ones")
            nc.gpsimd.dma_start(out=ones_s[:], in_=ones_d[:])
            ident = big.tile([128, 128], F32, tag="ident")
            make_identity(nc, ident[:])
            widx_acc = big.tile([128, NTILES], U32, tag="widx")
            loss_acc = big.tile([128, NTILES], F32, tag="lossacc")

            stash = {}

            def stage_a(t):
                xt_s = work.tile([128, 4, 128], BF16, tag="xt")
                for d in range(4):
                    nc.sync.dma_start(out=xt_s[:, d, :],
                                      in_=xt_d[d*128:(d+1)*128, t*128:(t+1)*128])
                x_s = work.tile([128, D], F32, tag="x")
                nc.sync.dma_start(out=x_s[:], in_=x_d[t*128:(t+1)*128, :])

                scores = scp.tile([128, K], F32, tag="scores")
                # d-outer matmuls over halves of 8 chunks: one weight load per
                # (stationary, half) instead of per chunk.
                for h in range(2):
                    pss = [psum.tile([128, 512], F32, tag="ps", name=f"ps{c}")
                           for c in range(8)]
                    for c in range(8):
                        k0 = (h * 8 + c) * 512
                        nc.tensor.matmul(pss[c][:], ones_s[:],
                                         crow_s[:, k0:k0+512],
                                         start=True, stop=False,
                                         skip_group_check=True)
                    for d in range(4):
                        for c in range(8):
                            k0 = (h * 8 + c) * 512
                            nc.tensor.matmul(pss[c][:], xt_s[:, d, :],
                                             ct_s[:, d, k0:k0+512],
                                             start=False, stop=(d == 3),
                                             skip_group_check=True)
                    for c in range(8):
                        k0 = (h * 8 + c) * 512
                        nc.scalar.activation(scores[:, k0:k0+512], pss[c][:],
                                             ACTF.Copy, scale=-1.0)

                if stages == 1:
                    if t == 0:
                        nc.sync.dma_start(out=lossp_d[:, :], in_=scores[:, :NTILES])
                    stash[t] = (x_s, None)
                    return
                t8 = small.tile([128, 8], F32, tag="t8")
                i8 = small.tile([128, 8], U32, tag="i8")
                nc.vector.max(out=t8[:], in_=scores[:])
                nc.vector.max_index(out=i8[:], in_max=t8[:], in_values=scores[:])
                if stages == 2:
                    if t == 0:
                        nc.sync.dma_start(out=lossp_d[:, :], in_=t8[:])
                        nc.sync.dma_start(out=widx_d[:, :], in_=i8[:])
                    stash[t] = (x_s, None)
                    return
                stash[t] = (x_s, i8)

            def stage_b(t):
                x_s, i8 = stash.pop(t)
                if i8 is None:
                    return
                cand = work.tile([128, NCAND, CBW], F32, tag="cand")
                for j in range(NCAND):
                    nc.gpsimd.indirect_dma_start(
                        out=cand[:, j, :], out_offset=None,
                        in_=cbn_d[:, :],
                        in_offset=bass.IndirectOffsetOnAxis(ap=i8[:, j:j+1], axis=0))

                # exact fp32 rescore: products on gpsimd, sums on ACT accum
                dots = small.tile([128, NCAND], F32, tag="dots")
                scr = work.tile([128, D], F32, tag="scr", bufs=1)
                for j in range(NCAND):
                    prod = work.tile([128, D], F32, tag="prod", name=f"prod{j}")
                    nc.gpsimd.tensor_tensor(out=prod[:], in0=cand[:, j, :D],
                                            in1=x_s[:], op=ALU.mult)
                    nc.scalar.activation(scr[:], prod[:], ACTF.Copy,
                                         accum_out=dots[:, j:j+1])
                xn = small.tile([128, 1], F32, tag="xn")
                nc.scalar.activation(scr[:], x_s[:], ACTF.Square, accum_out=xn[:])
                t1 = small.tile([128, NCAND], F32, tag="t1")
                nc.gpsimd.tensor_tensor(out=t1[:], in0=cand[:, :, D],
                                        in1=xn[:].broadcast_to([128, NCAND]),
                                        op=ALU.add)
                dots2 = small.tile([128, NCAND], F32, tag="dots2")
                nc.gpsimd.tensor_scalar_mul(dots2[:], dots[:], 2.0)
                r = small.tile([128, NCAND], F32, tag="r")
                nc.gpsimd.tensor_tensor(out=r[:], in0=t1[:], in1=dots2[:],
                                        op=ALU.subtract)

                # winner among candidates (ties -> lowest k), all on gpsimd
                rmin = small.tile([128, 1], F32, tag="rmin")
                nc.vector.tensor_reduce(out=rmin[:], in_=r[:], axis=AX, op=ALU.min)
                maskf = small.tile([128, NCAND], F32, tag="maskf")
                nc.vector.tensor_tensor(out=maskf[:], in0=r[:],
                                        in1=rmin[:].broadcast_to([128, NCAND]),
                                        op=ALU.is_le)
                idxf = small.tile([128, NCAND], F32, tag="idxf")
                nc.gpsimd.tensor_copy(out=idxf[:], in_=i8[:, :NCAND])
                ta = small.tile([128, NCAND], F32, tag="ta")
                nc.gpsimd.tensor_tensor(out=ta[:], in0=idxf[:], in1=maskf[:],
                                        op=ALU.mult)
                tb = small.tile([128, NCAND], F32, tag="tb")
                nc.gpsimd.tensor_scalar(out=tb[:], in0=maskf[:],
                                        scalar1=-65536.0, scalar2=65536.0,
                                        op0=ALU.mult, op1=ALU.add)
                sel = small.tile([128, NCAND], F32, tag="sel")
                nc.gpsimd.tensor_tensor(out=sel[:], in0=ta[:], in1=tb[:],
                                        op=ALU.add)
                wf = small.tile([128, 1], F32, tag="wf")
                nc.vector.tensor_reduce(out=wf[:], in_=sel[:], axis=AX, op=ALU.min)
                nc.vector.tensor_copy(out=widx_acc[:, t:t+1], in_=wf[:])

                zq = work.tile([128, CBW], F32, tag="zq")
                nc.gpsimd.indirect_dma_start(
                    out=zq[:], out_offset=None, in_=cbn_d[:, :],
                    in_offset=bass.IndirectOffsetOnAxis(ap=widx_acc[:, t:t+1], axis=0))
                tdiff = work.tile([128, D], F32, tag="tdiff")
                nc.gpsimd.tensor_tensor(out=tdiff[:], in0=zq[:, :D], in1=x_s[:],
                                        op=ALU.subtract)
                out_st = work.tile([128, D], F32, tag="outst")
                nc.gpsimd.tensor_tensor(out=out_st[:], in0=x_s[:], in1=tdiff[:],
                                        op=ALU.add)
                nc.scalar.activation(scr[:], tdiff[:], ACTF.Square,
                                     accum_out=loss_acc[:, t:t+1])

                for d in range(4):
                    tp = psum.tile([128, 128], F32, tag="ps")
                    nc.tensor.transpose(tp[:], out_st[:, d*128:(d+1)*128], ident[:])
                    oT = small.tile([128, 128], F32, tag="oT")
                    nc.scalar.copy(out=oT[:], in_=tp[:])
                    nc.sync.dma_start(
                        out=zqt_d[d*128:(d+1)*128, t*128:(t+1)*128], in_=oT[:])

            def pipelined():
                stage_a(0)
                for t in range(1, NTILES):
                    stage_a(t)
                    stage_b(t - 1)
                stage_b(NTILES - 1)

            if n_loop > 0:
                with tc.For_i(0, n_loop, 1):
                    pipelined()
            else:
                pipelined()

            if stages >= 3:
                nc.sync.dma_start(out=widx_d[:, :], in_=widx_acc[:])
                nc.sync.dma_start(out=lossp_d[:, :], in_=loss_acc[:])

    nc.finalize()
    return nc


def _prep_shared(codebook):
    import ml_dtypes
    k = np.arange(K)
    cnorm = (codebook.astype(np.float64) ** 2).sum(1).astype(np.float32)
    crow = np.stack([
        cnorm,
        ((k >> 7).astype(np.float32)) * 2.0 ** -7,
        ((k & 127).astype(np.float32)) * 2.0 ** -13,
        np.zeros(K, np.float32)]).astype(np.float32)
    ct_bf = np.ascontiguousarray((-2.0 * codebook.T).astype(ml_dtypes.bfloat16))
    cbn = np.zeros((K, CBW), np.float32)
    cbn[:, :512] = codebook
    cbn[:, 512] = cnorm
    ones = np.ones((4, 128), np.float32)
    return cbn, ct_bf, crow, ones


def kernel(z_e, codebook):
    import ml_dtypes
    from concourse.bass_utils import run_bass_kernel_spmd

    z_e = np.asarray(z_e, dtype=np.float32)
    codebook = np.asarray(codebook, dtype=np.float32)

    if "nc" not in _CACHE:
        _CACHE["nc"] = _build()
    nc = _CACHE["nc"]

    cbn, ct_bf, crow, ones = _prep_shared(codebook)
    in_maps = []
    for i in range(N_CORES):
        zc = z_e[4*i:4*(i+1)]                                   # (4, 512, 256)
        xt = zc.transpose(1, 0, 2).reshape(512, 1024)           # d-major
        x = np.ascontiguousarray(zc.transpose(0, 2, 1).reshape(1024, 512))
        in_maps.append({
            "xt": xt.astype(ml_dtypes.bfloat16),
            "x": x,
            "ct": ct_bf,
            "crow": crow,
            "ones": ones,
            "cbn": cbn,
        })

    res = run_bass_kernel_spmd(nc, in_maps, list(range(N_CORES)))

    zq_parts = []
    idx_parts = []
    loss_sum = 0.0
    for i in range(N_CORES):
        out = res.results[i]
        zq_parts.append(out["zqt"].reshape(512, 4, 256).transpose(1, 0, 2))
        idx_parts.append(out["widx"].transpose(1, 0).reshape(-1))  # n = t*128+p
        loss_sum += out["lossp"].sum(dtype=np.float64)

    z_q_out = np.ascontiguousarray(np.concatenate(zq_parts, axis=0),
                                   dtype=np.float32)             # (32, 512, 256)
    idx = np.concatenate(idx_parts).astype(np.int64)

    e_latent = np.float32(loss_sum / (B * T * EMBED_DIM))
    vq_loss = np.float32(ALPHA * BETA) * e_latent

    counts = np.bincount(idx, minlength=K).astype(np.float32)
    avg = counts / np.float32(B * T)
    ent = -(avg * np.log(avg + np.float32(1e-10), dtype=np.float32)).sum(
        dtype=np.float32)
    perplexity = np.exp(ent, dtype=np.float32)

    return z_q_out, np.float32(vq_loss), np.float32(perplexity)


# revision 15
# speedup vs baseline: 20.2843x; 20.2843x over previous
"""TRN2 Bass kernel for VQ codebook lookup (nn_BrainEncoder).

Contract: kernel(**inputs) takes FULL inputs
    z_e:      (32, 512, 256) f32   (b, d, t)
    codebook: (8192, 512) f32      (K, d)
returns (z_q_out (32,512,256) f32, vq_loss f32 scalar, perplexity f32 scalar)
exactly like the reference nn.Module.

Distribution: data-parallel over the flattened token dim N = 32*256 = 8192
across 8 NeuronCores (1024 tokens each); the codebook is replicated.

Per-core device kernel:
  - PE: dist'[n,k] = -2*x.c (bf16 matmuls) + |c_k|^2 + unique-k epsilon
        (one extra f32r matmul row) accumulated in PSUM fp32.
  - ACT: negate-copy PSUM -> SBUF scores.
  - DVE: top-8 scan + max_index -> candidate indices per token.
  - indirect-DMA gather of top-4 candidate rows (+their |c|^2).
  - exact fp32 rescore (DVE fused mult+accum) -> winner = exact fp32 argmin,
    ties to lowest k (matches jnp.argmin; the coarse bf16 pass only
    shortlists, the winner is decided at full fp32 precision).
  - winner row gather; straight-through out = z + (z_q - z) elementwise in
    fp32 (bit-identical formula to the reference); loss partials via ACT
    Square+accumulate; PE transposes the output back to d-major.
Host: shard/unshard, sum of per-core loss partials, bincount of the 8192
winner indices -> perplexity (the tiny O(K) scalar tail of the op).
"""

import numpy as np

ALPHA = 1.0
BETA = 0.25
B, EMBED_DIM, T = 32, 512, 256
K = 8192
D = 512
N_CORES = 8
NTILES = 8
NCHUNK = 16
NCAND = 4
CBW = 520

_CACHE = {}


def _build(n_loop=0, n_cores=N_CORES, stages=3, mm_order="c_outer",
           psum_bufs=8, ncand=NCAND, do_transpose=True):
    import concourse.bass as bass
    import concourse.mybir as mybir
    from concourse import bacc
    from concourse.tile import TileContext
    from concourse.masks import make_identity

    F32 = mybir.dt.float32
    F32R = mybir.dt.float32r
    BF16 = mybir.dt.bfloat16
    U32 = mybir.dt.uint32
    AX = mybir.AxisListType.X
    ALU = mybir.AluOpType
    ACTF = mybir.ActivationFunctionType

    nc = bacc.Bacc("TRN2", target_bir_lowering=False, debug=False,
                   num_devices=n_cores)
    xt_d = nc.declare_dram_parameter("xt", [512, 1024], BF16, isOutput=False)
    x_d = nc.declare_dram_parameter("x", [1024, 512], F32, isOutput=False)
    ct_d = nc.declare_dram_parameter("ct", [512, K], BF16, isOutput=False)
    crow_d = nc.declare_dram_parameter("crow", [4, K], F32, isOutput=False)
    ones_d = nc.declare_dram_parameter("ones", [4, 128], F32, isOutput=False)
    cbn_d = nc.declare_dram_parameter("cbn", [K, CBW], F32, isOutput=False)
    zqt_d = nc.declare_dram_parameter("zqt", [512, 1024], F32, isOutput=True)
    widx_d = nc.declare_dram_parameter("widx", [128, NTILES], U32, isOutput=True)
    lossp_d = nc.declare_dram_parameter("lossp", [128, NTILES], F32, isOutput=True)

    with TileContext(nc) as tc:
        with tc.tile_pool(name="big", bufs=1) as big, \
             tc.tile_pool(name="sc", bufs=2) as scp, \
             tc.tile_pool(name="work", bufs=2) as work, \
             tc.tile_pool(name="small", bufs=3) as small, \
             tc.tile_pool(name="psum", bufs=psum_bufs, space="PSUM") as psum:
            ct_s = big.tile([128, 4, K], BF16, tag="ct")
            for d in range(4):
                nc.sync.dma_start(out=ct_s[:, d, :], in_=ct_d[d*128:(d+1)*128, :])
            crow_s = big.tile([4, K], F32R, tag="crow")
            nc.gpsimd.dma_start(out=crow_s[:, :], in_=crow_d[:, :])
            ones_s = big.tile([4, 128], F32R, tag="ones")
            nc.gpsimd.dma_start(out=ones_s[:], in_=ones_d[:])
            ident = big.tile([128, 128], F32, tag="ident")
            make_identity(nc, ident[:])
            widx_acc = big.tile([128, NTILES], U32, tag="widx")
            loss_acc = big.tile([128, NTILES], F32, tag="lossacc")

            stash = {}

            def stage_a(t):
                xt_s = work.tile([128, 4, 128], BF16, tag="xt")
                for d in range(4):
                    nc.sync.dma_start(out=xt_s[:, d, :],
                                      in_=xt_d[d*128:(d+1)*128, t*128:(t+1)*128])
                x_s = work.tile([128, D], F32, tag="x")
                nc.sync.dma_start(out=x_s[:], in_=x_d[t*128:(t+1)*128, :])

                scores = scp.tile([128, K], F32, tag="scores")
                if mm_order == "d_outer":
                    # one weight load per (stationary, half of 8 chunks)
                    for h in range(2):
                        pss = [psum.tile([128, 512], F32, tag="ps", name=f"ps{c}")
                               for c in range(8)]
                        for c in range(8):
                            k0 = (h * 8 + c) * 512
                            nc.tensor.matmul(pss[c][:], ones_s[:],
                                             crow_s[:, k0:k0+512],
                                             start=True, stop=False,
                                             skip_group_check=True)
                        for d in range(4):
                            for c in range(8):
                                k0 = (h * 8 + c) * 512
                                nc.tensor.matmul(pss[c][:], xt_s[:, d, :],
                                                 ct_s[:, d, k0:k0+512],
                                                 start=False, stop=(d == 3),
                                                 skip_group_check=True)
                        for c in range(8):
                            k0 = (h * 8 + c) * 512
                            nc.scalar.activation(scores[:, k0:k0+512], pss[c][:],
                                                 ACTF.Copy, scale=-1.0)
                else:
                    for c in range(NCHUNK):
                        k0 = c * 512
                        ps = psum.tile([128, 512], F32, tag="ps")
                        nc.tensor.matmul(ps[:], ones_s[:], crow_s[:, k0:k0+512],
                                         start=True, stop=False,
                                         skip_group_check=True)
                        for d in range(4):
                            nc.tensor.matmul(ps[:], xt_s[:, d, :],
                                             ct_s[:, d, k0:k0+512],
                                             start=False, stop=(d == 3),
                                             skip_group_check=True)
                        nc.scalar.activation(scores[:, k0:k0+512], ps[:],
                                             ACTF.Copy, scale=-1.0)

                if stages == 1:
                    if t == 0:
                        nc.sync.dma_start(out=lossp_d[:, :], in_=scores[:, :NTILES])
                    stash[t] = (x_s, None)
                    return
                t8 = small.tile([128, 8], F32, tag="t8")
                i8 = small.tile([128, 8], U32, tag="i8")
                nc.vector.max(out=t8[:], in_=scores[:])
                nc.vector.max_index(out=i8[:], in_max=t8[:], in_values=scores[:])
                if stages == 2:
                    if t == 0:
                        nc.sync.dma_start(out=lossp_d[:, :], in_=t8[:])
                        nc.sync.dma_start(out=widx_d[:, :], in_=i8[:])
                    stash[t] = (x_s, None)
                    return
                stash[t] = (x_s, i8)

            def stage_b(t):
                x_s, i8 = stash.pop(t)
                if i8 is None:
                    return
                cand = work.tile([128, ncand, CBW], F32, tag="cand")
                for j in range(ncand):
                    nc.gpsimd.indirect_dma_start(
                        out=cand[:, j, :], out_offset=None,
                        in_=cbn_d[:, :],
                        in_offset=bass.IndirectOffsetOnAxis(ap=i8[:, j:j+1], axis=0))

                # exact fp32 rescore: products on gpsimd, sums on ACT accum
                dots = small.tile([128, ncand], F32, tag="dots")
                scr = work.tile([128, D], F32, tag="scr", bufs=1)
                for j in range(ncand):
                    prod = work.tile([128, D], F32, tag="prod", name=f"prod{j}")
                    nc.gpsimd.tensor_tensor(out=prod[:], in0=cand[:, j, :D],
                                            in1=x_s[:], op=ALU.mult)
                    nc.scalar.activation(scr[:], prod[:], ACTF.Copy,
                                         accum_out=dots[:, j:j+1])
                xn = small.tile([128, 1], F32, tag="xn")
                nc.scalar.activation(scr[:], x_s[:], ACTF.Square, accum_out=xn[:])
                t1 = small.tile([128, ncand], F32, tag="t1")
                nc.gpsimd.tensor_tensor(out=t1[:], in0=cand[:, :, D],
                                        in1=xn[:].broadcast_to([128, ncand]),
                                        op=ALU.add)
                dots2 = small.tile([128, ncand], F32, tag="dots2")
                nc.gpsimd.tensor_scalar_mul(dots2[:], dots[:], 2.0)
                r = small.tile([128, ncand], F32, tag="r")
                nc.gpsimd.tensor_tensor(out=r[:], in0=t1[:], in1=dots2[:],
                                        op=ALU.subtract)

                # winner among candidates (ties -> lowest k), all on gpsimd
                rmin = small.tile([128, 1], F32, tag="rmin")
                nc.vector.tensor_reduce(out=rmin[:], in_=r[:], axis=AX, op=ALU.min)
                maskf = small.tile([128, ncand], F32, tag="maskf")
                nc.vector.tensor_tensor(out=maskf[:], in0=r[:],
                                        in1=rmin[:].broadcast_to([128, ncand]),
                                        op=ALU.is_le)
                idxf = small.tile([128, ncand], F32, tag="idxf")
                nc.gpsimd.tensor_copy(out=idxf[:], in_=i8[:, :ncand])
                ta = small.tile([128, ncand], F32, tag="ta")
                nc.gpsimd.tensor_tensor(out=ta[:], in0=idxf[:], in1=maskf[:],
                                        op=ALU.mult)
                tb = small.tile([128, ncand], F32, tag="tb")
                nc.gpsimd.tensor_scalar(out=tb[:], in0=maskf[:],
                                        scalar1=-65536.0, scalar2=65536.0,
                                        op0=ALU.mult, op1=ALU.add)
                sel = small.tile([128, ncand], F32, tag="sel")
                nc.gpsimd.tensor_tensor(out=sel[:], in0=ta[:], in1=tb[:],
                                        op=ALU.add)
                wf = small.tile([128, 1], F32, tag="wf")
                nc.vector.tensor_reduce(out=wf[:], in_=sel[:], axis=AX, op=ALU.min)
                nc.vector.tensor_copy(out=widx_acc[:, t:t+1], in_=wf[:])

                zq = work.tile([128, CBW], F32, tag="zq")
                nc.gpsimd.indirect_dma_start(
                    out=zq[:], out_offset=None, in_=cbn_d[:, :],
                    in_offset=bass.IndirectOffsetOnAxis(ap=widx_acc[:, t:t+1], axis=0))
                tdiff = work.tile([128, D], F32, tag="tdiff")
                nc.gpsimd.tensor_tensor(out=tdiff[:], in0=zq[:, :D], in1=x_s[:],
                                        op=ALU.subtract)
                out_st = work.tile([128, D], F32, tag="outst")
                nc.gpsimd.tensor_tensor(out=out_st[:], in0=x_s[:], in1=tdiff[:],
                                        op=ALU.add)
                nc.scalar.activation(scr[:], tdiff[:], ACTF.Square,
                                     accum_out=loss_acc[:, t:t+1])

                if do_transpose:
                    for d in range(4):
                        tp = psum.tile([128, 128], F32, tag="ps")
                        nc.tensor.transpose(tp[:], out_st[:, d*128:(d+1)*128],
                                            ident[:])
                        oT = small.tile([128, 128], F32, tag="oT")
                        nc.scalar.copy(out=oT[:], in_=tp[:])
                        nc.sync.dma_start(
                            out=zqt_d[d*128:(d+1)*128, t*128:(t+1)*128], in_=oT[:])
                else:
                    # timing-ablation only: same byte count, layout ignored
                    nc.sync.dma_start(out=zqt_d[:, t*128:(t+1)*128],
                                      in_=out_st[:])

            def pipelined():
                stage_a(0)
                for t in range(1, NTILES):
                    stage_a(t)
                    stage_b(t - 1)
                stage_b(NTILES - 1)

            if n_loop > 0:
                with tc.For_i(0, n_loop, 1):
                    pipelined()
            else:
                pipelined()

            if stages >= 3:
                nc.sync.dma_start(out=widx_d[:, :], in_=widx_acc[:])
                nc.sync.dma_start(out=lossp_d[:, :], in_=loss_acc[:])

    nc.finalize()
    return nc


def _prep_shared(codebook):
    import ml_dtypes
    k = np.arange(K)
    cnorm = (codebook.astype(np.float64) ** 2).sum(1).astype(np.float32)
    crow = np.stack([
        cnorm,
        ((k >> 7).astype(np.float32)) * 2.0 ** -7,
        ((k & 127).astype(np.float32)) * 2.0 ** -13,
        np.zeros(K, np.float32)]).astype(np.float32)
    ct_bf = np.ascontiguousarray((-2.0 * codebook.T).astype(ml_dtypes.bfloat16))
    cbn = np.zeros((K, CBW), np.float32)
    cbn[:, :512] = codebook
    cbn[:, 512] = cnorm
    ones = np.ones((4, 128), np.float32)
    return cbn, ct_bf, crow, ones


def kernel(z_e, codebook):
    import ml_dtypes
    from concourse.bass_utils import run_bass_kernel_spmd

    z_e = np.asarray(z_e, dtype=np.float32)
    codebook = np.asarray(codebook, dtype=np.float32)

    if "nc" not in _CACHE:
        _CACHE["nc"] = _build()
    nc = _CACHE["nc"]

    cbn, ct_bf, crow, ones = _prep_shared(codebook)
    in_maps = []
    for i in range(N_CORES):
        zc = z_e[4*i:4*(i+1)]                                   # (4, 512, 256)
        xt = zc.transpose(1, 0, 2).reshape(512, 1024)           # d-major
        x = np.ascontiguousarray(zc.transpose(0, 2, 1).reshape(1024, 512))
        in_maps.append({
            "xt": xt.astype(ml_dtypes.bfloat16),
            "x": x,
            "ct": ct_bf,
            "crow": crow,
            "ones": ones,
            "cbn": cbn,
        })

    res = run_bass_kernel_spmd(nc, in_maps, list(range(N_CORES)))

    zq_parts = []
    idx_parts = []
    loss_sum = 0.0
    for i in range(N_CORES):
        out = res.results[i]
        zq_parts.append(out["zqt"].reshape(512, 4, 256).transpose(1, 0, 2))
        idx_parts.append(out["widx"].transpose(1, 0).reshape(-1))  # n = t*128+p
        loss_sum += out["lossp"].sum(dtype=np.float64)

    z_q_out = np.ascontiguousarray(np.concatenate(zq_parts, axis=0),
                                   dtype=np.float32)             # (32, 512, 256)
    idx = np.concatenate(idx_parts).astype(np.int64)

    e_latent = np.float32(loss_sum / (B * T * EMBED_DIM))
    vq_loss = np.float32(ALPHA * BETA) * e_latent

    counts = np.bincount(idx, minlength=K).astype(np.float32)
    avg = counts / np.float32(B * T)
    ent = -(avg * np.log(avg + np.float32(1e-10), dtype=np.float32)).sum(
        dtype=np.float32)
    perplexity = np.exp(ent, dtype=np.float32)

    return z_q_out, np.float32(vq_loss), np.float32(perplexity)
